# revision 7
# baseline (speedup 1.0000x reference)
"""Trainium2 Bass kernel for the MANE multi-view SGNS embedding loss.

Data-parallel over batch across 8 NeuronCores, tables replicated.  All
embedding-row fetches go through the high-throughput ant `dma_gather`
(thousands of descriptors per instruction) instead of per-128-row indirect
DMAs.  dma_gather takes int16 indices, so the 200K-row vocab is bucketed
host-side into 32768-row windows; gathered rows land packed, and each row's
center vector is fetched in the same packed order from a small per-core DRAM
center table (built on device in phase A, int16-addressable).  Pad slots
read a reserved zero-center row so they contribute exactly log(1/2), which
the host corrects; bucket overflows (rare) are computed exactly on host.
Dots run as elementwise multiply+reduce on DVE (bf16), log-sigmoid +
per-term reduction on the scalar engine, per-core [128, 30] partials are
combined on the host (scalar all-reduce).
"""

from contextlib import ExitStack

import numpy as np

import concourse.bacc as bacc
import concourse.tile as tile
from concourse import mybir
from concourse.bass_utils import run_bass_kernel_spmd

# ---------------------------------------------------------------- problem dims
V, N, D = 3, 200000, 128
B, K = 32768, 10
NCORES = 8
P = 128
T = 3 + 2 * V * (V - 1)     # 15 terms
BC = B // NCORES            # 4096 batch items per core
VD = V * D                  # interleaved row: 3 views x D

CH = 32768                  # vocab bucket width (int16-addressable)
NCH = (N + CH - 1) // CH    # 7 buckets
CHW = [min(CH, N - q * CH) for q in range(NCH)]

NGRP = 4                    # groups per term
RG = BC * K // NGRP         # 10240 negative rows per (term, group)

PADS_B = [1792] * 6 + [256]          # per-bucket padded counts (neg)
BLK_B = [p // 128 for p in PADS_B]
NBLK = sum(BLK_B)                    # 86 blocks -> 11008 slots per (t, g)
OFS_B = np.cumsum([0] + BLK_B).tolist()

PADS_P = [768] * 6 + [128]           # per-bucket padded counts (pos)
BLK_P = [p // 128 for p in PADS_P]
PBLK = sum(BLK_P)                    # 37 blocks -> 4736 slots
OFS_P = np.cumsum([0] + BLK_P).tolist()

PADS_A = [896] * 6 + [128]           # phase-A center buckets
BLK_A = [p // 128 for p in PADS_A]
ABLK = sum(BLK_A)                    # 43 blocks -> 5504 positions per view
OFS_A = np.cumsum([0] + BLK_A).tolist()
CZERO = ABLK * 128                   # per-view zero-center row
CVS = CZERO + 128                    # per-view stride in cen_tab rows

F32 = mybir.dt.float32
I16 = mybir.dt.int16
BF16 = mybir.dt.bfloat16
NPBF = mybir.dt.np(BF16)

PAIRS = [(j, i) for j in range(V) for i in range(V) if i != j]
# per term: (neg/pos table is neigh_T, j = view slice of rows, center view)
TERM_INFO = ([(True, i, i) for i in range(V)]
             + [(False, j, i) for (j, i) in PAIRS]
             + [(True, j, i) for (j, i) in PAIRS])

# idx16 column layout of the [128, NI16] int16 input
A_COLS = sum(PADS_A) // 16                    # per view
PROW_COLS = sum(PADS_P) // 16
PCEN_COLS = PBLK * 128 // 16
PPOS_COLS = PROW_COLS + PCEN_COLS             # per (view, type)
NROW_COLS = sum(PADS_B) // 16
NCEN_COLS = NBLK * 128 // 16
NNEG_COLS = NROW_COLS + NCEN_COLS             # per (t, g)
A0 = 0
P0 = A0 + V * A_COLS
N0 = P0 + 6 * PPOS_COLS
NI16 = N0 + T * NGRP * NNEG_COLS

LN2 = float(np.log(2.0))


def _snake(a):
    """int array [n] (n % 16 == 0) -> [128, n//16] int16 snake, replicated."""
    s = a.reshape(-1, 16).T.astype(np.int16)
    return np.tile(s, (8, 1))


def emit(nc, tc, ctx, node_ap, neigh_ap, idx_ap, acc_ap):
    dram_pool = ctx.enter_context(tc.tile_pool(name="dram", bufs=1,
                                               space="DRAM"))
    io_pool = ctx.enter_context(tc.tile_pool(name="io", bufs=2))
    g2_pool = ctx.enter_context(tc.tile_pool(name="g2", bufs=2))
    g1_pool = ctx.enter_context(tc.tile_pool(name="g1", bufs=1))
    x_pool = ctx.enter_context(tc.tile_pool(name="x", bufs=1))
    act_pool = ctx.enter_context(tc.tile_pool(name="act", bufs=2))
    out_pool = ctx.enter_context(tc.tile_pool(name="out", bufs=1))

    CEN = dram_pool.tile([V * CVS, D], BF16)
    cen_ap = CEN[:]

    # zero the per-view pad-center rows
    zt = x_pool.tile([P, D], BF16, tag="zt")
    nc.vector.memset(zt[:], 0.0)
    for i in range(V):
        nc.sync.dma_start(cen_ap[i * CVS + CZERO:i * CVS + CZERO + P], zt[:])

    XN = x_pool.tile([P, T * NGRP * NBLK], F32, tag="xn")
    XPn = [x_pool.tile([P, PBLK * V], F32, tag=f"xpn{i}", name=f"xpn{i}")
           for i in range(V)]
    XPg = [x_pool.tile([P, PBLK * V], F32, tag=f"xpg{i}", name=f"xpg{i}")
           for i in range(V)]
    ACC = out_pool.tile([P, 2 * T], F32)

    # ---- phase A: gather per-batch centers (packed) -> cen_tab in DRAM
    for i in range(V):
        ait = io_pool.tile([P, A_COLS], I16, tag="ait")
        nc.sync.dma_start(ait[:], idx_ap[:, A0 + i * A_COLS:
                                         A0 + (i + 1) * A_COLS])
        pc = g1_pool.tile([P, ABLK * D], BF16, tag="pc")
        for q in range(NCH):
            nc.gpsimd.dma_gather(
                out_ap=pc[:, OFS_A[q] * D:OFS_A[q + 1] * D]
                .rearrange("p (x e) -> p x e", e=D),
                in_ap=node_ap[q * CH:q * CH + CHW[q], i * D:(i + 1) * D],
                idxs_ap=ait[:, OFS_A[q] * 8:OFS_A[q] * 8 + PADS_A[q] // 16],
                num_idxs=PADS_A[q], num_idxs_reg=PADS_A[q],
                elem_size=D, elem_step=VD, single_packet=False)
        nc.sync.dma_start(
            cen_ap[i * CVS:i * CVS + ABLK * P]
            .rearrange("(x p) e -> p x e", p=P),
            pc[:].rearrange("p (x e) -> p x e", e=D))

    # ---- positives: packed full-row gathers + packed center fetch
    for i in range(V):
        for (ti, tbl, XP) in ((0, node_ap, XPn[i]), (1, neigh_ap, XPg[i])):
            base = P0 + (i * 2 + ti) * PPOS_COLS
            pit = io_pool.tile([P, PPOS_COLS], I16, tag="pit")
            nc.sync.dma_start(pit[:], idx_ap[:, base:base + PPOS_COLS])
            PT = g1_pool.tile([P, PBLK * VD], BF16, tag="pt")
            for q in range(NCH):
                nc.gpsimd.dma_gather(
                    out_ap=PT[:, OFS_P[q] * VD:OFS_P[q + 1] * VD]
                    .rearrange("p (x e) -> p x e", e=VD),
                    in_ap=tbl[q * CH:q * CH + CHW[q], :],
                    idxs_ap=pit[:, OFS_P[q] * 8:
                                OFS_P[q] * 8 + PADS_P[q] // 16],
                    num_idxs=PADS_P[q], num_idxs_reg=PADS_P[q],
                    elem_size=VD, elem_step=VD, single_packet=False)
            CT2 = g1_pool.tile([P, PBLK * D], BF16, tag="ct2")
            nc.gpsimd.dma_gather(
                out_ap=CT2[:].rearrange("p (x e) -> p x e", e=D),
                in_ap=cen_ap[i * CVS:(i + 1) * CVS],
                idxs_ap=pit[:, PROW_COLS:PROW_COLS + PCEN_COLS],
                num_idxs=PBLK * 128, num_idxs_reg=PBLK * 128,
                elem_size=D, elem_step=D, single_packet=False)
            cb = (CT2[:].rearrange("p (x e) -> p x e", e=D)
                  .unsqueeze(2).to_broadcast([P, PBLK, V, D]))
            nc.vector.tensor_tensor(
                out=PT[:].rearrange("p (x v e) -> p x v e", v=V, e=D),
                in0=PT[:].rearrange("p (x v e) -> p x v e", v=V, e=D),
                in1=cb, op=mybir.AluOpType.mult)
            nc.vector.tensor_reduce(
                out=XP[:], in_=PT[:].rearrange("p (m e) -> p m e", e=D),
                axis=mybir.AxisListType.X, op=mybir.AluOpType.add)

    # ---- negatives: per (term, group) packed gathers
    for t in range(T):
        useg, j, iv = TERM_INFO[t]
        tbl = neigh_ap if useg else node_ap
        for g in range(NGRP):
            base = N0 + (t * NGRP + g) * NNEG_COLS
            nit = io_pool.tile([P, NNEG_COLS], I16, tag="nit")
            nc.sync.dma_start(nit[:], idx_ap[:, base:base + NNEG_COLS])
            RT = g2_pool.tile([P, NBLK * D], BF16, tag="rt")
            for q in range(NCH):
                nc.gpsimd.dma_gather(
                    out_ap=RT[:, OFS_B[q] * D:OFS_B[q + 1] * D]
                    .rearrange("p (x e) -> p x e", e=D),
                    in_ap=tbl[q * CH:q * CH + CHW[q], j * D:(j + 1) * D],
                    idxs_ap=nit[:, OFS_B[q] * 8:
                                OFS_B[q] * 8 + PADS_B[q] // 16],
                    num_idxs=PADS_B[q], num_idxs_reg=PADS_B[q],
                    elem_size=D, elem_step=VD, single_packet=False)
            CT = g2_pool.tile([P, NBLK * D], BF16, tag="ct")
            nc.gpsimd.dma_gather(
                out_ap=CT[:].rearrange("p (x e) -> p x e", e=D),
                in_ap=cen_ap[iv * CVS:(iv + 1) * CVS],
                idxs_ap=nit[:, NROW_COLS:NROW_COLS + NCEN_COLS],
                num_idxs=NBLK * 128, num_idxs_reg=NBLK * 128,
                elem_size=D, elem_step=D, single_packet=False)
            nc.vector.tensor_tensor(out=RT[:], in0=RT[:], in1=CT[:],
                                    op=mybir.AluOpType.mult)
            nc.vector.tensor_reduce(
                out=XN[:, (t * NGRP + g) * NBLK:(t * NGRP + g + 1) * NBLK],
                in_=RT[:].rearrange("p (m e) -> p m e", e=D),
                axis=mybir.AxisListType.X, op=mybir.AluOpType.add)

    # ---- log-sigmoid + accumulate: ACC[:, 0:T] = neg sums, [T:2T] = pos
    for t in range(T):
        useg, j, iv = TERM_INFO[t]
        sgn = act_pool.tile([P, NGRP * NBLK], F32, tag="sgn")
        nc.scalar.activation(
            out=sgn[:], in_=XN[:, t * NGRP * NBLK:(t + 1) * NGRP * NBLK],
            func=mybir.ActivationFunctionType.Sigmoid, scale=-1.0)
        spn = act_pool.tile([P, NGRP * NBLK], F32, tag="spn")
        nc.scalar.activation(
            out=spn[:], in_=sgn[:], func=mybir.ActivationFunctionType.Ln,
            accum_out=ACC[:, t:t + 1])
        XP = XPg[iv] if useg else XPn[iv]
        pos_in = XP[:].rearrange("p (m v) -> p m v", v=V)[:, :, j]
        sgp = act_pool.tile([P, PBLK], F32, tag="sgp")
        nc.scalar.activation(out=sgp[:], in_=pos_in,
                             func=mybir.ActivationFunctionType.Sigmoid)
        spp = act_pool.tile([P, PBLK], F32, tag="spp")
        nc.scalar.activation(out=spp[:], in_=sgp[:],
                             func=mybir.ActivationFunctionType.Ln,
                             accum_out=ACC[:, T + t:T + t + 1])

    nc.sync.dma_start(acc_ap, ACC[:])


def build_bass():
    nc = bacc.Bacc("TRN2", target_bir_lowering=False, debug=False,
                   enable_asserts=False)
    node_t = nc.dram_tensor("node_t", [N, VD], BF16, kind="ExternalInput")
    neigh_t = nc.dram_tensor("neigh_t", [N, VD], BF16, kind="ExternalInput")
    idx16 = nc.dram_tensor("idx16", [P, NI16], I16, kind="ExternalInput")
    acc_out = nc.dram_tensor("acc", [P, 2 * T], F32, kind="ExternalOutput")
    with tile.TileContext(nc) as tc, ExitStack() as ctx:
        emit(nc, tc, ctx, node_t.ap(), neigh_t.ap(), idx16.ap(), acc_out.ap())
    nc.compile()
    return nc


_NC_CACHE = {}


def _get_nc():
    if "nc" not in _NC_CACHE:
        _NC_CACHE["nc"] = build_bass()
    return _NC_CACHE["nc"]


def _bucketize(rows, pads):
    """rows int64 [n] -> (idx16 stream, slot_of_row [n] (-1 dropped),
    dropped row positions, npad)."""
    q = rows >> 15
    local = rows & (CH - 1)
    order = np.argsort(q, kind="stable")
    counts = np.bincount(q, minlength=NCH)
    stream, dropped = [], []
    slot = np.full(rows.shape[0], -1, dtype=np.int64)
    pos_ofs = 0
    o = 0
    npad = 0
    for qq in range(NCH):
        c = int(counts[qq])
        take = min(c, pads[qq])
        sel = order[o:o + take]
        stream.append(local[sel])
        stream.append(np.zeros(pads[qq] - take, dtype=np.int64))
        npad += pads[qq] - take
        slot[sel] = pos_ofs + np.arange(take)
        if c > take:
            dropped.extend(order[o + take:o + c].tolist())
        o += c
        pos_ofs += pads[qq]
    return np.concatenate(stream), slot, dropped, npad


def host_prep(count, shuffle_indices, nodes_idx, neigh_idx,
              neg_idx1, neg_idx2, neg_idx3, node_W, neigh_W,
              n_cores=NCORES):
    """Per-core input maps + host-side corrections. Pure numpy."""
    c0 = int(count)
    sh = np.asarray(shuffle_indices)[:, c0:c0 + B].astype(np.int64)
    nodes_sel = np.take_along_axis(
        np.asarray(nodes_idx).astype(np.int64), sh, axis=1)
    neigh_sel = np.take_along_axis(
        np.asarray(neigh_idx).astype(np.int64), sh, axis=1)
    neg1 = np.asarray(neg_idx1).astype(np.int64)[:, :B]
    neg2 = np.asarray(neg_idx2).astype(np.int64)[:, :, :B]
    neg3 = np.asarray(neg_idx3).astype(np.int64)[:, :, :B]

    node_T = np.ascontiguousarray(
        np.asarray(node_W).astype(NPBF).transpose(1, 0, 2)).reshape(N, VD)
    neigh_T = np.ascontiguousarray(
        np.asarray(neigh_W).astype(NPBF).transpose(1, 0, 2)).reshape(N, VD)
    nTf = node_T.astype(np.float64).reshape(N, V, D)
    gTf = neigh_T.astype(np.float64).reshape(N, V, D)

    neg_list = ([neg1[i] for i in range(V)]
                + [neg2[j, i] for (j, i) in PAIRS]
                + [neg3[j, i] for (j, i) in PAIRS])   # T x [B, K]

    in_maps, extras = [], []
    for core in range(n_cores):
        sl = slice(core * BC, (core + 1) * BC)
        extra = np.zeros((T, 2), dtype=np.float64)
        cols = np.zeros((P, NI16), dtype=np.int16)

        # phase A: centers per view
        pos_of_b, centers_f = [], []
        for i in range(V):
            rows = nodes_sel[i, sl]
            stream, slot, dropped, _ = _bucketize(rows, PADS_A)
            assert not dropped, "phase-A bucket overflow"
            cols[:, A0 + i * A_COLS:A0 + (i + 1) * A_COLS] = _snake(stream)
            pos_of_b.append(slot)
            centers_f.append(nTf[rows, i, :])

        # positives
        for i in range(V):
            for (ti, tblf, sel_rows) in (
                    (0, nTf, nodes_sel[i, sl]),
                    (1, gTf, neigh_sel[i, sl])):
                base = P0 + (i * 2 + ti) * PPOS_COLS
                stream, slot, dropped, npad = _bucketize(sel_rows, PADS_P)
                cols[:, base:base + PROW_COLS] = _snake(stream)
                cidx = np.full(PBLK * 128, CZERO, dtype=np.int64)
                ok = np.nonzero(slot >= 0)[0]
                cidx[slot[ok]] = pos_of_b[i][ok]
                cols[:, base + PROW_COLS:base + PPOS_COLS] = _snake(cidx)
                terms = [(t, TERM_INFO[t][1]) for t in range(T)
                         if (TERM_INFO[t][0] == (ti == 1)
                             and TERM_INFO[t][2] == i)]
                for (t, jj) in terms:
                    extra[t, 1] += LN2 * npad
                    for bi in dropped:
                        x = float(tblf[sel_rows[bi], jj, :]
                                  @ centers_f[i][bi])
                        extra[t, 1] += float(-np.logaddexp(0.0, -x) + LN2)

        # negatives
        for t in range(T):
            useg, j, iv = TERM_INFO[t]
            tblf = gTf if useg else nTf
            rows_t = neg_list[t][sl].reshape(-1)
            bvec = np.repeat(np.arange(BC), K)
            for g in range(NGRP):
                gsl = slice(g * RG, (g + 1) * RG)
                rows = rows_t[gsl]
                bfor = bvec[gsl]
                base = N0 + (t * NGRP + g) * NNEG_COLS
                stream, slot, dropped, npad = _bucketize(rows, PADS_B)
                cols[:, base:base + NROW_COLS] = _snake(stream)
                cidx = np.full(NBLK * 128, CZERO, dtype=np.int64)
                ok = np.nonzero(slot >= 0)[0]
                cidx[slot[ok]] = pos_of_b[iv][bfor[ok]]
                cols[:, base + NROW_COLS:base + NNEG_COLS] = _snake(cidx)
                extra[t, 0] += LN2 * npad
                for ridx in dropped:
                    x = float(tblf[rows[ridx], j, :]
                              @ centers_f[iv][bfor[ridx]])
                    extra[t, 0] += float(-np.logaddexp(0.0, x) + LN2)

        in_maps.append({"node_t": node_T, "neigh_t": neigh_T,
                        "idx16": cols})
        extras.append(extra)
    return in_maps, extras


def host_combine(acc_list, extras, hyp1, hyp2, b=B):
    s = np.zeros(T, dtype=np.float64)
    for a, extra in zip(acc_list, extras):
        a = np.asarray(a, dtype=np.float64).sum(axis=0)
        s += a[:T] + a[T:2 * T] + extra[:, 0] + extra[:, 1]
    term_val = s / b
    cost1 = term_val[0:3].mean()
    cost2 = float(np.asarray(hyp1).reshape(-1)[0]) * term_val[3:9].sum() / 6.0
    cost3 = float(np.asarray(hyp2).reshape(-1)[0]) * term_val[9:15].sum() / 6.0
    return np.array(-(cost1 + cost2 + cost3) / 3.0, dtype=np.float32)


def kernel(count, shuffle_indices, nodes_idx, neigh_idx,
           neg_idx1, neg_idx2, neg_idx3, node_W, neigh_W, hyp1, hyp2):
    in_maps, extras = host_prep(count, shuffle_indices, nodes_idx, neigh_idx,
                                neg_idx1, neg_idx2, neg_idx3, node_W, neigh_W)
    nc = _get_nc()
    res = run_bass_kernel_spmd(nc, in_maps, core_ids=list(range(NCORES)))
    acc_list = [r["acc"] for r in res.results]
    return host_combine(acc_list, extras, hyp1, hyp2)
